# revision 3
# baseline (speedup 1.0000x reference)
"""Trainium2 Bass kernel for a 3-layer GatedGraphConv (Devign-style) GNN.

Strategy (data-parallel over graphs, 8 NeuronCores):
  - 256 graphs -> 32 graphs per core. Nodes are re-laid-out per device into
    fixed per-graph slots (GMAX slots per graph) so the compiled program is
    identical across cores (SPMD) while per-core data differs.
  - Edges are owned by the core that owns their dst node. Aggregation
    (scatter-add) = indirect-DMA row gather from a replicated node-feature
    table + one-hot matmul accumulation into PSUM windows.
  - Per-layer node features are exchanged with an AllGather collective.
  - GRU gates are computed feature-major ([128 feat, nodes]) with stationary
    weights; x-side weights are folded with the per-layer GGNN matrix on the
    host (Wc = ggnn_W[i] @ W_ih.T), and the input projection is folded into
    the embedding table (T = embed @ W_in.T + b_in) so layer-0 features are
    a pure gather.
"""

import numpy as np

H = 128
NUM_LAYERS = 3
NUM_GRAPHS = 256
N_CORES = 8
GRAPHS_PER = NUM_GRAPHS // N_CORES
GATHER_BLK = 16          # edge tiles per gather chunk (16*128 = 2048 edges)
WIN_BLKS = 4             # dst blocks (128 nodes) per PSUM window (512 nodes)

LAST_EXEC_NS = None


def _round_up(x, m):
    return (x + m - 1) // m * m


def _host_prep(x_lex, edge_index, batch, embed_table, W_in, b_in, ggnn_W,
               gru_W_ih, gru_W_hh, gru_b_ih, gru_b_hh,
               cls_W1, cls_b1, cls_W2, cls_b2):
    """All numpy. Returns (meta, shared_inputs, per_core_inputs)."""
    x_lex = np.asarray(x_lex).astype(np.int64)
    src = np.asarray(edge_index[0]).astype(np.int64)
    dst = np.asarray(edge_index[1]).astype(np.int64)
    batch = np.asarray(batch).astype(np.int64)
    n_nodes = x_lex.shape[0]
    vocab = embed_table.shape[0]

    # ---- graph / node layout -------------------------------------------
    gb = np.searchsorted(batch, np.arange(NUM_GRAPHS + 1))
    gsize = gb[1:] - gb[:-1]
    GMAX = _round_up(max(int(gsize.max()), 1), 16)
    NPAD = _round_up(GRAPHS_PER * GMAX, 512)
    NB = NPAD // 128

    g_of_node = batch
    core_of_node = g_of_node // GRAPHS_PER
    slot_in_graph = np.arange(n_nodes) - gb[g_of_node]
    local_slot = (g_of_node % GRAPHS_PER) * GMAX + slot_in_graph
    gslot = core_of_node * NPAD + local_slot

    # ---- folded embedding table ----------------------------------------
    taug = np.concatenate(
        [np.asarray(embed_table, np.float32) @ np.asarray(W_in, np.float32).T
         + np.asarray(b_in, np.float32)[None, :],
         np.zeros((1, H), np.float32)], axis=0)      # [vocab+1, H]

    emb_idx = np.full((N_CORES, 128, NB), vocab, np.int32)
    p_ = local_slot % 128
    j_ = local_slot // 128
    emb_idx[core_of_node, p_, j_] = x_lex.astype(np.int32)

    pmask = np.full((N_CORES, NPAD), -1.0e30, np.float32)
    pmask[core_of_node, local_slot] = 0.0

    # ---- edge layout ----------------------------------------------------
    e_core = core_of_node[dst]
    e_dstslot = local_slot[dst]
    e_srcg = gslot[src]

    e_block = e_dstslot // 128
    counts = np.zeros((N_CORES, NB), np.int64)
    np.add.at(counts, (e_core, e_block), 1)
    tiles_b = np.maximum(1, -(-counts.max(axis=0) // 128))
    T = int(tiles_b.sum())
    T = _round_up(T, GATHER_BLK)

    tile_block = np.zeros(T, np.int64)
    tile_start = np.zeros(T, bool)
    tile_stop = np.zeros(T, bool)
    t = 0
    block_t0 = np.zeros(NB + 1, np.int64)
    for b in range(NB):
        block_t0[b] = t
        nb_t = int(tiles_b[b])
        tile_block[t:t + nb_t] = b
        tile_start[t] = True
        tile_stop[t + nb_t - 1] = True
        t += nb_t
    block_t0[NB] = t
    if t < T:
        tile_block[t:] = NB - 1
        tile_stop[t - 1] = False
        tile_stop[T - 1] = True

    eidx = np.zeros((N_CORES, 128, T), np.int32)
    eslot = np.full((N_CORES, 128, T), -1.0, np.float32)
    order = np.lexsort((e_dstslot, e_core))
    ec, es, eg = e_core[order], e_dstslot[order], e_srcg[order]
    cb = np.searchsorted(ec, np.arange(N_CORES + 1))
    for c in range(N_CORES):
        s0, s1 = cb[c], cb[c + 1]
        blk = es[s0:s1] // 128
        bbound = np.searchsorted(blk, np.arange(NB + 1))
        for b in range(NB):
            k0, k1 = bbound[b], bbound[b + 1]
            cnt = k1 - k0
            if cnt == 0:
                continue
            t0 = block_t0[b]
            pos = np.arange(cnt)
            eidx[c, pos % 128, t0 + pos // 128] = eg[s0 + k0:s0 + k1].astype(np.int32)
            eslot[c, pos % 128, t0 + pos // 128] = (es[s0 + k0:s0 + k1] % 128).astype(np.float32)

    # ---- folded weights -------------------------------------------------
    ggnn_W = np.asarray(ggnn_W, np.float32)
    W_ih = np.asarray(gru_W_ih, np.float32)
    W_hh = np.asarray(gru_W_hh, np.float32)
    b_ih = np.asarray(gru_b_ih, np.float32)
    b_hh = np.asarray(gru_b_hh, np.float32)
    wc = np.stack([ggnn_W[i] @ W_ih.T for i in range(NUM_LAYERS)])  # [L,128,384]
    whht = np.ascontiguousarray(W_hh.T)                             # [128,384]

    shared = {
        "taug": taug,
        "iota": np.broadcast_to(np.arange(128, dtype=np.float32), (128, 128)).copy(),
        "ident": np.eye(128, dtype=np.float32),
        "wc": wc.astype(np.float32),
        "whht": whht.astype(np.float32),
        "b_r": (b_ih[0:H] + b_hh[0:H]).reshape(H, 1).astype(np.float32),
        "b_z": (b_ih[H:2 * H] + b_hh[H:2 * H]).reshape(H, 1).astype(np.float32),
        "b_in_n": b_ih[2 * H:3 * H].reshape(H, 1).astype(np.float32),
        "b_hn": b_hh[2 * H:3 * H].reshape(H, 1).astype(np.float32),
        "clsw1t": np.ascontiguousarray(np.asarray(cls_W1, np.float32).T),  # [128,64]
        "clsb1": np.asarray(cls_b1, np.float32).reshape(-1, 1),
        "clsw2t": np.ascontiguousarray(np.asarray(cls_W2, np.float32).T),  # [64,1]
        "clsb2": np.asarray(cls_b2, np.float32).reshape(1, 1),
    }
    per_core = [
        {"embidx": emb_idx[c], "eidx": eidx[c], "eslot": eslot[c],
         "pmask": np.broadcast_to(pmask[c][None, :], (128, NPAD)).copy()}
        for c in range(N_CORES)
    ]
    meta = dict(NPAD=NPAD, NB=NB, T=T, GMAX=GMAX, VOCAB1=vocab + 1,
                tile_block=tile_block, tile_start=tile_start,
                tile_stop=tile_stop)
    return meta, shared, per_core


def _build_program(meta):
    import concourse.bass as bass
    import concourse.mybir as mybir
    from concourse.tile import TileContext

    f32 = mybir.dt.float32
    i32 = mybir.dt.int32
    NPAD, NB, T = meta["NPAD"], meta["NB"], meta["T"]
    GMAX = meta["GMAX"]
    VOCAB1 = meta["VOCAB1"]
    tile_block = meta["tile_block"]
    tile_start = meta["tile_start"]
    tile_stop = meta["tile_stop"]
    NWIN = NB // WIN_BLKS
    assert NB % WIN_BLKS == 0

    nc = bass.Bass(num_devices=N_CORES)

    # ---- I/O -----------------------------------------------------------
    taug = nc.dram_tensor("taug", [VOCAB1, H], f32, kind="ExternalInput")
    iota_in = nc.dram_tensor("iota", [128, 128], f32, kind="ExternalInput")
    ident_in = nc.dram_tensor("ident", [128, 128], f32, kind="ExternalInput")
    wc_in = nc.dram_tensor("wc", [NUM_LAYERS, H, 3 * H], f32, kind="ExternalInput")
    whht_in = nc.dram_tensor("whht", [H, 3 * H], f32, kind="ExternalInput")
    b_r_in = nc.dram_tensor("b_r", [H, 1], f32, kind="ExternalInput")
    b_z_in = nc.dram_tensor("b_z", [H, 1], f32, kind="ExternalInput")
    b_in_n_in = nc.dram_tensor("b_in_n", [H, 1], f32, kind="ExternalInput")
    b_hn_in = nc.dram_tensor("b_hn", [H, 1], f32, kind="ExternalInput")
    clsw1t_in = nc.dram_tensor("clsw1t", [H, 64], f32, kind="ExternalInput")
    clsb1_in = nc.dram_tensor("clsb1", [64, 1], f32, kind="ExternalInput")
    clsw2t_in = nc.dram_tensor("clsw2t", [64, 1], f32, kind="ExternalInput")
    clsb2_in = nc.dram_tensor("clsb2", [1, 1], f32, kind="ExternalInput")
    embidx_in = nc.dram_tensor("embidx", [128, NB], i32, kind="ExternalInput")
    eidx_in = nc.dram_tensor("eidx", [128, T], i32, kind="ExternalInput")
    eslot_in = nc.dram_tensor("eslot", [128, T], f32, kind="ExternalInput")
    pmask_in = nc.dram_tensor("pmask", [128, NPAD], f32, kind="ExternalInput")

    pooled_out = nc.dram_tensor("pooledT", [128, GRAPHS_PER], f32, kind="ExternalOutput")
    logits_out = nc.dram_tensor("logits", [1, GRAPHS_PER], f32, kind="ExternalOutput")

    hsend = nc.dram_tensor("hsend", [NPAD, H], f32, kind="Internal")
    table = nc.dram_tensor("table", [N_CORES * NPAD, H], f32, kind="Internal",
                           addr_space="Shared")
    rg = [list(range(N_CORES))]

    with TileContext(nc) as tc:
        with tc.tile_pool(name="const", bufs=1) as cpool, \
             tc.tile_pool(name="hpool", bufs=1) as hpool, \
             tc.tile_pool(name="gath", bufs=2) as gpool, \
             tc.tile_pool(name="work", bufs=2) as wpool, \
             tc.tile_pool(name="stage", bufs=4) as spool, \
             tc.tile_pool(name="psA", bufs=2, space="PSUM") as psA, \
             tc.tile_pool(name="psB", bufs=2, space="PSUM") as psB, \
             tc.tile_pool(name="psC", bufs=1, space="PSUM") as psC, \
             tc.tile_pool(name="psT", bufs=2, space="PSUM") as psT:

            # ---- constants into SBUF ----------------------------------
            iota_t = cpool.tile([128, 128], f32)
            ident_t = cpool.tile([128, 128], f32)
            wc_t = cpool.tile([128, NUM_LAYERS * 3 * H], f32)
            whht_t = cpool.tile([128, 3 * H], f32)
            b_r_t = cpool.tile([128, 1], f32)
            b_z_t = cpool.tile([128, 1], f32)
            b_in_n_t = cpool.tile([128, 1], f32)
            b_hn_t = cpool.tile([128, 1], f32)
            clsw1t_t = cpool.tile([128, 64], f32)
            clsb1_t = cpool.tile([64, 1], f32)
            clsw2t_t = cpool.tile([64, 1], f32)
            clsb2_t = cpool.tile([1, 1], f32)
            embidx_t = cpool.tile([128, NB], i32)
            eidx_t = cpool.tile([128, T], i32)
            eslot_t = cpool.tile([128, T], f32)

            nc.sync.dma_start(out=iota_t[:], in_=iota_in[:])
            nc.sync.dma_start(out=ident_t[:], in_=ident_in[:])
            nc.sync.dma_start(
                out=wc_t[:].rearrange("p (l o) -> p l o", l=NUM_LAYERS),
                in_=wc_in[:].rearrange("l p o -> p l o"))
            nc.sync.dma_start(out=whht_t[:], in_=whht_in[:])
            nc.sync.dma_start(out=b_r_t[:], in_=b_r_in[:])
            nc.sync.dma_start(out=b_z_t[:], in_=b_z_in[:])
            nc.sync.dma_start(out=b_in_n_t[:], in_=b_in_n_in[:])
            nc.sync.dma_start(out=b_hn_t[:], in_=b_hn_in[:])
            nc.sync.dma_start(out=clsw1t_t[:], in_=clsw1t_in[:])
            nc.sync.dma_start(out=clsb1_t[:], in_=clsb1_in[:])
            nc.sync.dma_start(out=clsw2t_t[:], in_=clsw2t_in[:])
            nc.sync.dma_start(out=clsb2_t[:], in_=clsb2_in[:])
            nc.sync.dma_start(out=embidx_t[:], in_=embidx_in[:])
            nc.sync.dma_start(out=eidx_t[:], in_=eidx_in[:])
            nc.sync.dma_start(out=eslot_t[:], in_=eslot_in[:])

            def wc_l(layer):     # [128, 384] lhsT slab for layer
                return wc_t[:, layer * 3 * H:(layer + 1) * 3 * H]

            h_t = hpool.tile([128, NPAD], f32)        # h transposed (feat-major)

            def gather_rows(dst_tile, idx_ap, n_cols):
                """dst_tile[p, j, :] = taug/table row idx_ap[p, j].
                Emitted as per-column [128,1] indirect DMAs (HW-validated form).
                """
                for j in range(n_cols):
                    nc.gpsimd.indirect_dma_start(
                        out=dst_tile[:, j, :],
                        out_offset=None,
                        in_=gather_rows.src[:],
                        in_offset=bass.IndirectOffsetOnAxis(
                            ap=idx_ap[:, j:j + 1], axis=0),
                    )

            # ---- embed: h0 = taug[x]; write hsend (node-major) + h_t ---
            EMB_BLK = 8
            gather_rows.src = taug
            for c in range(NB // EMB_BLK + (1 if NB % EMB_BLK else 0)):
                b0 = c * EMB_BLK
                nb = min(EMB_BLK, NB - b0)
                if nb <= 0:
                    break
                ge = gpool.tile([128, EMB_BLK, H], f32, tag="embg")
                gather_rows(ge, embidx_t[:, b0:b0 + nb], nb)
                nc.sync.dma_start(
                    out=hsend[b0 * 128:(b0 + nb) * 128, :].rearrange(
                        "(j p) f -> p j f", p=128),
                    in_=ge[:, :nb, :])
                for b in range(nb):
                    tp = psT.tile([128, 128], f32, space="PSUM", tag="tp")
                    nc.tensor.transpose(tp[:], ge[:, b, :], ident_t[:])
                    nc.scalar.copy(out=h_t[:, (b0 + b) * 128:(b0 + b + 1) * 128],
                                   in_=tp[:])

            # window -> [tile range) in the global tile order
            win_t0 = [None] * NWIN
            win_t1 = [None] * NWIN
            for t in range(T):
                w = int(tile_block[t]) // WIN_BLKS
                if win_t0[w] is None:
                    win_t0[w] = t
                win_t1[w] = t + 1
            # make ranges cover all tiles contiguously
            cur = 0
            for w in range(NWIN):
                if win_t0[w] is None:
                    win_t0[w] = cur
                    win_t1[w] = cur
                win_t0[w] = cur
                cur = max(cur, win_t1[w])
                win_t1[w] = cur

            gather_rows.src = table
            NCHUNK = T // GATHER_BLK

            for layer in range(NUM_LAYERS):
                # -- AllGather h (node-major) into replicated table
                nc.gpsimd.collective_compute(
                    "AllGather", mybir.AluOpType.bypass,
                    ins=[hsend[:]], outs=[table[:]], replica_groups=rg)

                # -- gather chunks + B build
                gts = []
                bts = []
                for g in range(NCHUNK):
                    gt = gpool.tile([128, GATHER_BLK, H], f32, tag="edgeg")
                    gather_rows(gt, eidx_t[:, g * GATHER_BLK:(g + 1) * GATHER_BLK],
                                GATHER_BLK)
                    bt = wpool.tile([128, GATHER_BLK, 128], f32, tag="bmat")
                    nc.vector.tensor_tensor(
                        out=bt[:],
                        in0=eslot_t[:, g * GATHER_BLK:(g + 1) * GATHER_BLK, None]
                            .to_broadcast([128, GATHER_BLK, 128]),
                        in1=iota_t[:, None, :].to_broadcast([128, GATHER_BLK, 128]),
                        op=mybir.AluOpType.is_equal)
                    gts.append(gt)
                    bts.append(bt)

                # -- per-window: matmul accumulate, flush, GRU, (transpose)
                for w in range(NWIN):
                    t0, t1 = win_t0[w], win_t1[w]
                    aggps = psA.tile([128, 512], f32, space="PSUM", tag="aggps")
                    if t0 == t1:
                        nc.vector.memset(aggps[:], 0.0)
                    for t in range(t0, t1):
                        g, j = divmod(t, GATHER_BLK)
                        blk = int(tile_block[t])
                        rel = blk % WIN_BLKS
                        nc.tensor.matmul(
                            out=aggps[:, rel * 128:(rel + 1) * 128],
                            lhsT=gts[g][:, j, :],
                            rhs=bts[g][:, j, :],
                            start=bool(tile_start[t]),
                            stop=bool(tile_stop[t]),
                        )
                    agg_sb = wpool.tile([128, 512], f32, tag="aggsb")
                    nc.scalar.copy(out=agg_sb[:], in_=aggps[:])

                    # -- GRU on this window's 512 nodes
                    hs = h_t[:, w * 512:(w + 1) * 512]
                    wcl = wc_l(layer)
                    ps_rz = psB.tile([128, 512], f32, space="PSUM", tag="ps_rz")
                    nc.tensor.matmul(out=ps_rz[:], lhsT=wcl[:, 0:128],
                                     rhs=agg_sb[:], start=True, stop=False)
                    nc.tensor.matmul(out=ps_rz[:], lhsT=whht_t[:, 0:128],
                                     rhs=hs, start=False, stop=True)
                    r_sb = wpool.tile([128, 512], f32, tag="r")
                    nc.scalar.activation(r_sb[:], ps_rz[:],
                                         mybir.ActivationFunctionType.Sigmoid,
                                         bias=b_r_t[:, 0:1])
                    ps_z = psB.tile([128, 512], f32, space="PSUM", tag="ps_rz")
                    nc.tensor.matmul(out=ps_z[:], lhsT=wcl[:, 128:256],
                                     rhs=agg_sb[:], start=True, stop=False)
                    nc.tensor.matmul(out=ps_z[:], lhsT=whht_t[:, 128:256],
                                     rhs=hs, start=False, stop=True)
                    z_sb = wpool.tile([128, 512], f32, tag="z")
                    nc.scalar.activation(z_sb[:], ps_z[:],
                                         mybir.ActivationFunctionType.Sigmoid,
                                         bias=b_z_t[:, 0:1])
                    ps_gn = psC.tile([128, 512], f32, space="PSUM", tag="ps_gn")
                    nc.tensor.matmul(out=ps_gn[:], lhsT=wcl[:, 256:384],
                                     rhs=agg_sb[:], start=True, stop=True)
                    ps_hn = psC.tile([128, 512], f32, space="PSUM", tag="ps_hn")
                    nc.tensor.matmul(out=ps_hn[:], lhsT=whht_t[:, 256:384],
                                     rhs=hs, start=True, stop=True)
                    t1_sb = wpool.tile([128, 512], f32, tag="t1")
                    nc.scalar.activation(t1_sb[:], ps_hn[:],
                                         mybir.ActivationFunctionType.Identity,
                                         bias=b_hn_t[:, 0:1])
                    t2_sb = wpool.tile([128, 512], f32, tag="t2")
                    nc.vector.tensor_mul(out=t2_sb[:], in0=r_sb[:], in1=t1_sb[:])
                    t3_sb = wpool.tile([128, 512], f32, tag="t3")
                    nc.vector.tensor_add(out=t3_sb[:], in0=t2_sb[:], in1=ps_gn[:])
                    n_sb = wpool.tile([128, 512], f32, tag="n")
                    nc.scalar.activation(n_sb[:], t3_sb[:],
                                         mybir.ActivationFunctionType.Tanh,
                                         bias=b_in_n_t[:, 0:1])
                    t4_sb = wpool.tile([128, 512], f32, tag="t4")
                    nc.vector.tensor_sub(out=t4_sb[:], in0=hs, in1=n_sb[:])
                    t5_sb = wpool.tile([128, 512], f32, tag="t5")
                    nc.vector.tensor_mul(out=t5_sb[:], in0=z_sb[:], in1=t4_sb[:])
                    nc.vector.tensor_add(out=hs, in0=n_sb[:], in1=t5_sb[:])

                    # -- write back node-major h for next layer's AllGather
                    if layer < NUM_LAYERS - 1:
                        for b in range(WIN_BLKS):
                            blk = w * WIN_BLKS + b
                            tp = psT.tile([128, 128], f32, space="PSUM", tag="tp")
                            nc.tensor.transpose(
                                tp[:], h_t[:, blk * 128:(blk + 1) * 128],
                                ident_t[:])
                            st = spool.tile([128, 128], f32, tag="hstage")
                            nc.scalar.copy(out=st[:], in_=tp[:])
                            nc.sync.dma_start(
                                out=hsend[blk * 128:(blk + 1) * 128, :],
                                in_=st[:])

            # ---- pooling: per graph masked max --------------------------
            pooled_t = cpool.tile([128, GRAPHS_PER], f32)
            for g in range(GRAPHS_PER):
                s0 = g * GMAX
                pm = spool.tile([128, GMAX], f32, tag="pm")
                nc.sync.dma_start(out=pm[:], in_=pmask_in[:, s0:s0 + GMAX])
                hm = spool.tile([128, GMAX], f32, tag="hm")
                nc.vector.tensor_add(out=hm[:], in0=h_t[:, s0:s0 + GMAX],
                                     in1=pm[:])
                nc.vector.reduce_max(pooled_t[:, g:g + 1], hm[:],
                                     axis=mybir.AxisListType.X)
            ge_t = cpool.tile([128, GRAPHS_PER], f32)
            nc.vector.tensor_scalar(out=ge_t[:], in0=pooled_t[:],
                                    scalar1=-1.0e29, scalar2=None,
                                    op0=mybir.AluOpType.is_ge)
            nc.vector.tensor_mul(out=pooled_t[:], in0=pooled_t[:], in1=ge_t[:])
            nc.sync.dma_start(out=pooled_out[:], in_=pooled_t[:])

            # ---- classifier --------------------------------------------
            ps1 = psC.tile([64, GRAPHS_PER], f32, space="PSUM", tag="ps_gn")
            nc.tensor.matmul(out=ps1[:], lhsT=clsw1t_t[:], rhs=pooled_t[:],
                             start=True, stop=True)
            z1 = cpool.tile([64, GRAPHS_PER], f32)
            nc.scalar.activation(z1[:], ps1[:],
                                 mybir.ActivationFunctionType.Relu,
                                 bias=clsb1_t[:, 0:1])
            ps2 = psC.tile([1, GRAPHS_PER], f32, space="PSUM", tag="ps_hn")
            nc.tensor.matmul(out=ps2[:], lhsT=clsw2t_t[:], rhs=z1[:],
                             start=True, stop=True)
            lg = cpool.tile([1, GRAPHS_PER], f32)
            nc.vector.tensor_add(out=lg[:], in0=ps2[:],
                                 in1=clsb2_t[:].to_broadcast([1, GRAPHS_PER]))
            nc.sync.dma_start(out=logits_out[:], in_=lg[:])

    return nc


def kernel(**inputs):
    global LAST_EXEC_NS
    import os
    import sys
    sys.path.insert(0, "/root/problem/work")
    try:
        from waitfix import split_excess_waits
    except ImportError:
        split_excess_waits = _split_excess_waits
    from concourse.bass_utils import run_bass_kernel_spmd

    meta, shared, per_core = _host_prep(**inputs)
    nc = _build_program(meta)
    split_excess_waits(nc)

    trace = bool(os.environ.get("KERNEL_TRACE"))
    if trace:
        try:
            import axhook
            axhook.install()
        except Exception:
            trace = False
    in_maps = [dict(shared, **pc) for pc in per_core]
    res = run_bass_kernel_spmd(nc, in_maps, core_ids=list(range(N_CORES)),
                               trace=trace)
    LAST_EXEC_NS = res.exec_time_ns

    logits = np.zeros((NUM_GRAPHS, 1), np.float32)
    pooled = np.zeros((NUM_GRAPHS, H), np.float32)
    for c in range(N_CORES):
        r = res.results[c]
        pooled[c * GRAPHS_PER:(c + 1) * GRAPHS_PER, :] = r["pooledT"].T
        logits[c * GRAPHS_PER:(c + 1) * GRAPHS_PER, 0] = r["logits"][0]
    return logits, pooled


def _split_excess_waits(nc):
    """Split >capacity sync waits into preceding NOPs (walrus allows 1 wait
    per instruction, 2 for EventSemaphore)."""
    import concourse.mybir as mybir
    for fn in nc.m.functions:
        for bb in fn.blocks:
            insts = list(bb.instructions)
            out = []
            changed = False
            for inst in insts:
                si = inst.sync_info
                waits = list(si.on_wait) if si is not None and si.on_wait else []
                cap = 2 if isinstance(inst, mybir.InstEventSemaphore) else 1
                if len(waits) > cap:
                    changed = True
                    for j, w in enumerate(waits[cap:]):
                        out.append(mybir.InstNoOp(
                            name=f"{inst.name}-waitsplit-{j}",
                            engine=inst.engine, ins=[], outs=[],
                            sync_info=mybir.SyncInfo(on_wait=[w], on_update=[])))
                    si.on_wait = waits[:cap]
                out.append(inst)
            if changed:
                bb.instructions.clear()
                for i in out:
                    bb.add_instruction(i)
    return nc
